# revision 49
# baseline (speedup 1.0000x reference)
"""Trainium2 Bass kernel for nn_Block_3599182594921 (gnn_message_passing).

Pure data parallel over batch B=32 across 8 NeuronCores (4 samples/core).

v2 design:
- LayerNorm affines (n1/n2/n3) folded into the adjacent weights host-side;
  the device only standardizes ((x-mean)*rstd).
- The gather table stores [standardized(384) | mean | std] bf16 rows, so
  stage-2's per-chunk LN disappears (LN commutes with row gather); raw
  rows are reconstructed on-chip (x*std, mean folded into the proj psum
  via a ones-matmul).
- Table is sample-major (row = s*4224 + core*528 + t) so the AllGather is
  split into 4 per-sample pieces that overlap stage-1 compute of later
  samples.
- Stage-1/2 q,k computed with per-head packed [q_h|k_h] weights -> full
  128-partition matmuls; softmax denominator folded into the attention
  matmul via a ones-augmented V (row 64 of the psum = den).
- Engine ops write upper partition halves directly (offset-64 writes are
  legal); no DMA round-trips for odd heads.
- Pooling GELU deferred out of the chunk loop to avoid Exp<->Gelu
  activation-table thrash.
"""

import contextlib

import numpy as np
import ml_dtypes

import concourse.bacc as bacc
import concourse.bass as bass
import concourse.tile as tile
from concourse import mybir
from concourse.bass_utils import run_bass_kernel_spmd
from concourse.masks import make_identity

f32 = mybir.dt.float32
f32r = mybir.dt.float32r
bf16 = mybir.dt.bfloat16
i32 = mybir.dt.int32
AX = mybir.AxisListType
OP = mybir.AluOpType
ACTF = mybir.ActivationFunctionType

N_CORES = 8
B, G, S, T, C, GK, H, HD = 32, 512, 128, 16, 384, 32, 6, 64
BPC = B // N_CORES          # 4 samples per core
N = T + G                   # 528 tokens per sample in stage 1
TBLS = N_CORES * N          # 4224 table rows per sample-slice
TBL = BPC * TBLS            # 16896 table rows total
R = 16                      # adapter bottleneck
H4 = 4 * C                  # 1536
RW = C + 4                  # 388 table row: [standardized | mean | std | pad]
NEG = -100000.0
SCALE = HD ** -0.5

TQ = 264                    # stage-1 moving-dim chunk (2 per sample)
TKS = [128, 128, 128, 128, 16]


def _std_fm(nc, sb3, ppT, xT, ntok, ones_col, ones_row, out):
    """Standardize over C of feature-major xT [128, 3, ntok] -> out.

    (x - mean) / sqrt(var + eps); no affine (folded into weights).
    Stats via PE ones-matmuls (partition reduction).
    """
    nq = (ntok + TQ - 1) // TQ
    for q in range(nq):
        q0 = q * TQ
        qn = min(TQ, ntok - q0)
        ps = ppT.tile([1, TQ], f32, tag="t", name="lnp1")
        for ch in range(3):
            nc.tensor.matmul(out=ps[:, :qn], lhsT=ones_col[:, :1],
                             rhs=xT[:, ch, q0:q0 + qn],
                             start=(ch == 0), stop=(ch == 2))
        mean = sb3.tile([1, TQ], f32, tag="ln_mean", name="lnmean", bufs=2)
        nc.scalar.mul(out=mean[:, :qn], in_=ps[:, :qn], mul=1.0 / C)
        ps2 = ppT.tile([1, TQ], f32, tag="t", name="lnp2")
        for ch in range(3):
            sq = sb3.tile([128, TQ], f32, tag="ln_sq", name="lnsq", bufs=2)
            nc.vector.tensor_tensor(out=sq[:, :qn], in0=xT[:, ch, q0:q0 + qn],
                                    in1=xT[:, ch, q0:q0 + qn], op=OP.mult)
            nc.tensor.matmul(out=ps2[:, :qn], lhsT=ones_col[:, :1],
                             rhs=sq[:, :qn], start=(ch == 0), stop=(ch == 2))
        var = sb3.tile([1, TQ], f32, tag="ln_var", name="lnvar", bufs=2)
        # var = E[x^2] - mean^2 ; rstd = 1/sqrt(var+eps)
        nc.vector.scalar_tensor_tensor(out=var[:, :qn], in0=mean[:, :qn],
                                       scalar=-1.0, in1=mean[:, :qn],
                                       op0=OP.mult, op1=OP.mult)
        nc.vector.scalar_tensor_tensor(out=var[:, :qn], in0=ps2[:, :qn],
                                       scalar=1.0 / C, in1=var[:, :qn],
                                       op0=OP.mult, op1=OP.add)
        nc.vector.tensor_scalar(out=var[:, :qn], in0=var[:, :qn], scalar1=1e-5,
                                scalar2=None, op0=OP.add)
        nc.scalar.activation(out=var[:, :qn], in_=var[:, :qn], func=ACTF.Sqrt)
        rr = sb3.tile([1, TQ], f32r, tag="ln_rr", name="lnrr", bufs=2)
        with nc.allow_low_precision(reason="f32r rstd (full fp32 width)"):
            nc.vector.reciprocal(out=rr[:, :qn], in_=var[:, :qn])
        mr = sb3.tile([1, TQ], f32r, tag="ln_mr", name="lnmr", bufs=2)
        nc.vector.tensor_tensor(out=mr[:, :qn], in0=mean[:, :qn],
                                in1=rr[:, :qn], op=OP.mult)
        b_r = ppT.tile([128, TQ], f32, tag="t", name="lnbr")
        nc.tensor.matmul(out=b_r[:, :qn], lhsT=ones_row[:1, :],
                         rhs=rr[:1, :qn], start=True, stop=True)
        b_m = ppT.tile([128, TQ], f32, tag="t", name="lnbm")
        nc.tensor.matmul(out=b_m[:, :qn], lhsT=ones_row[:1, :],
                         rhs=mr[:1, :qn], start=True, stop=True)
        for ch in range(3):
            t = sb3.tile([128, TQ], f32, tag="ln_t", name="lnt", bufs=2)
            nc.vector.tensor_tensor(out=t[:, :qn], in0=xT[:, ch, q0:q0 + qn],
                                    in1=b_r[:, :qn], op=OP.mult)
            nc.vector.tensor_tensor(out=out[:, ch, q0:q0 + qn], in0=t[:, :qn],
                                    in1=b_m[:, :qn], op=OP.subtract)


def build(debug_outputs=(), repeat=1):
    nc = bacc.Bacc("TRN2", target_bir_lowering=False, debug=False,
                   num_devices=N_CORES)
    dd = {}

    def din(name, shape, dtype=f32):
        dd[name] = nc.dram_tensor(name, shape, dtype, kind="ExternalInput")
        return dd[name]

    din("xinT", [BPC, 3, 128, N])
    din("maskT", [BPC, N, N], bf16)
    din("idx2", [128, BPC * S * GK // 128], i32)   # [128, 128]
    din("cidx2", [128, BPC], i32)
    din("c1T", [BPC, 3, G])
    din("n1sq", [BPC, 1, G])
    din("c2T", [BPC, 3, S])
    din("n2sq", [BPC, S, 1])
    din("wqk1", [3, 128, 768])
    din("qb1", [128, 6])
    din("wv1", [3, 128, C])
    din("wprojT", [3, 128, C])
    din("bproj", [3, 128])
    din("wfc1T", [3, 128, H4], bf16)
    din("bfc1", [12, 128])
    din("wfc2T", [12, 128, C], bf16)
    din("bfc2", [3, 128])
    din("waddT", [3, 128, R])
    din("adb", [R, 1])
    din("waduT", [R, C])
    din("adub", [3, 128])
    din("wa1dT", [3, 128, R])
    din("a1db", [R, 1])
    din("wa1uT", [R, C])
    din("a1ub", [3, 128])
    din("wqk2", [3, 128, 768], bf16)
    din("qb2", [128, 6])
    din("wv2", [3, 128, C], bf16)
    din("wa1projT", [3, 128, C], bf16)
    din("ba1proj", [3, 128])
    din("bnscale", [3, 128]), din("bnbias", [3, 128])
    din("gate", [1, 1])
    din("blockmask", [128, 512], bf16)
    y = nc.dram_tensor("y", [BPC, G, C], f32, kind="ExternalOutput")
    dbg = {}
    for dn, shape in debug_outputs:
        dbg[dn] = nc.dram_tensor(dn, shape, f32, kind="ExternalOutput")

    with tile.TileContext(nc) as tc:
        ctx = contextlib.ExitStack()
        with ctx:
            dram = ctx.enter_context(tc.tile_pool(name="dram", bufs=1,
                                                  space="DRAM"))
            wp = ctx.enter_context(tc.tile_pool(name="wp", bufs=1))

            in_b = [dram.tile([N, RW], bf16, name=f"in_b{s}")
                    for s in range(BPC)]
            table = dram.tile([TBL, RW], bf16)
            x2d = dram.tile([BPC, 128, 3, N], f32)

            # ---- load weights (staging pool freed after this block) ----
            with tc.tile_pool(name="wst", bufs=1) as wst:
                def load_w(name, chunks, width, dtype):
                    src = dd[name]
                    if dtype == bf16:
                        w = wp.tile([128, chunks, width], bf16,
                                    name=f"w_{name}")
                        nc.sync.dma_start(out=w[:], in_=src[:, :, :].rearrange(
                            "a p x -> p a x"))
                        return w
                    stg = wst.tile([128, chunks, width], f32, tag="wstage",
                                   name=f"stg_{name}")
                    nc.sync.dma_start(out=stg[:], in_=src[:, :, :].rearrange(
                        "a p x -> p a x"))
                    w = wp.tile([128, chunks, width], dtype, name=f"w_{name}")
                    nc.vector.tensor_copy(out=w[:], in_=stg[:])
                    return w

                wqk1 = load_w("wqk1", 3, 768, f32r)
                wv1 = load_w("wv1", 3, C, f32r)
                wproj = load_w("wprojT", 3, C, f32r)
                wfc1 = load_w("wfc1T", 3, H4, bf16)
                wfc2 = load_w("wfc2T", 12, C, bf16)
                wadd = load_w("waddT", 3, R, f32r)
                wa1d = load_w("wa1dT", 3, R, f32r)
                wqk2 = load_w("wqk2", 3, 768, bf16)
                wv2 = load_w("wv2", 3, C, bf16)
                wa1proj = load_w("wa1projT", 3, C, bf16)

                wadu_f = wst.tile([R, C], f32, tag="usmall", name="wadu_f")
                nc.sync.dma_start(out=wadu_f[:], in_=dd["waduT"][:, :])
                wadu = wp.tile([R, C], f32r)
                nc.vector.tensor_copy(out=wadu[:], in_=wadu_f[:])
                wa1u_f = wst.tile([R, C], f32, tag="usmall2", name="wa1u_f")
                nc.sync.dma_start(out=wa1u_f[:], in_=dd["wa1uT"][:, :])
                wa1u = wp.tile([R, C], f32r)
                nc.vector.tensor_copy(out=wa1u[:], in_=wa1u_f[:])

                cols = {}
                for cn, p in [("bproj", 3), ("bfc1", 12), ("bfc2", 3),
                              ("adub", 3), ("a1ub", 3), ("ba1proj", 3),
                              ("bnscale", 3), ("bnbias", 3)]:
                    t = wp.tile([128, p], f32, name=f"col_{cn}")
                    nc.sync.dma_start(out=t[:],
                                      in_=dd[cn][:, :].rearrange("a p -> p a"))
                    cols[cn] = t
                qb1c = wp.tile([128, 6], f32, name="qb1c")
                nc.sync.dma_start(out=qb1c[:], in_=dd["qb1"][:, :])
                qb2c = wp.tile([128, 6], f32, name="qb2c")
                nc.sync.dma_start(out=qb2c[:], in_=dd["qb2"][:, :])
                adb_c = wp.tile([R, 1], f32, name="adb_c")
                nc.sync.dma_start(out=adb_c[:], in_=dd["adb"][:, :])
                a1db_c = wp.tile([R, 1], f32, name="a1db_c")
                nc.sync.dma_start(out=a1db_c[:], in_=dd["a1db"][:, :])
                gate_c = wp.tile([128, 1], f32)
                nc.sync.dma_start(out=gate_c[:],
                                  in_=dd["gate"][:, :].to_broadcast([128, 1]))
                bmask = wp.tile([128, 512], bf16)
                nc.sync.dma_start(out=bmask[:], in_=dd["blockmask"][:, :])
                idx2 = wp.tile([128, BPC * S * GK // 128], i32, name="idx2")
                nc.sync.dma_start(out=idx2[:], in_=dd["idx2"][:, :])
                cidx2 = wp.tile([128, BPC], i32, name="cidx2")
                nc.sync.dma_start(out=cidx2[:], in_=dd["cidx2"][:, :])

            ones_col_f = wp.tile([128, 1], f32)
            nc.vector.memset(ones_col_f[:], 1.0)
            ones_row_f = wp.tile([1, 128], f32)
            nc.vector.memset(ones_row_f[:], 1.0)
            ones_row = wp.tile([1, 128], f32r)
            nc.vector.tensor_copy(out=ones_row[:], in_=ones_row_f[:])
            ones_row_b = wp.tile([1, 128], bf16)
            nc.vector.memset(ones_row_b[:], 1.0)
            ones_col = wp.tile([128, 1], f32r)
            nc.vector.tensor_copy(out=ones_col[:], in_=ones_col_f[:])
            eps_c = wp.tile([128, 1], f32)
            nc.vector.memset(eps_c[:], 1e-5)
            ident = wp.tile([128, 128], f32)
            make_identity(nc, ident)
            ident_b = wp.tile([128, 128], bf16)
            nc.vector.tensor_copy(out=ident_b[:], in_=ident[:])

            for rep in range(repeat):
                # per-rep Shared collective outputs (single-writer rule)
                tbl_s = [dram.tile([TBLS, RW], bf16, addr_space="Shared",
                                   name=f"tbl{rep}_{s}") for s in range(BPC)]
                # ================= STAGE 1 =================
                with tc.tile_pool(name="sb1", bufs=1) as sb1, \
                     tc.tile_pool(name="sb3", bufs=3) as sb3, \
                     tc.tile_pool(name="pp1", bufs=2, space="PSUM") as ppT, \
                     tc.tile_pool(name="pp1a", bufs=4, space="PSUM") as ppP, \
                     tc.tile_pool(name="pp1s", bufs=2, space="PSUM") as ppS:
                    for s in range(BPC):
                        x0T = sb1.tile([128, 3, N], f32, tag="x0T", bufs=2)
                        nc.sync.dma_start(
                            out=x0T[:], in_=dd["xinT"][s, :, :, :].rearrange(
                                "a p x -> p a x"))
                        xh1 = sb1.tile([128, 3, N], f32r, tag="xh1", bufs=2)
                        _std_fm(nc, sb3, ppT, x0T, N, ones_col_f, ones_row,
                                xh1)
                        # head-pair packing: slot hp holds heads 2hp (parts
                        # 0:64) and 2hp+1 (parts 64:128); q and k separate
                        # tiles so score matmuls see matching base partitions
                        qkT = sb1.tile([128, 6, N], bf16, tag="qkT", bufs=2)
                        for qk in range(2):       # 0=q, 1=k
                            for hp in range(3):
                                for tq in range(2):
                                    t0 = tq * TQ
                                    ps = ppT.tile([128, TQ], f32, tag="t",
                                                  name="qkp")
                                    for ch in range(3):
                                        nc.tensor.matmul(
                                            out=ps[:],
                                            lhsT=wqk1[:, ch,
                                                      qk * C + hp * 128:
                                                      qk * C + (hp + 1) * 128],
                                            rhs=xh1[:, ch, t0:t0 + TQ],
                                            start=(ch == 0), stop=(ch == 2))
                                    nc.scalar.activation(
                                        out=qkT[:, qk * 3 + hp, t0:t0 + TQ],
                                        in_=ps[:], func=ACTF.Identity,
                                        bias=qb1c[:, qk * 3 + hp:
                                                  qk * 3 + hp + 1], scale=1.0)
                        # v, ones-augmented: [tok, j, h, 0:64]=v, [...,64]=1
                        v1 = sb1.tile([128, 5, H, HD + 1], bf16, tag="v1",
                                      bufs=2)
                        nc.vector.memset(v1[:, :, :, HD:HD + 1], 1.0)
                        for j, tk in enumerate(TKS):
                            t0 = j * 128
                            ps = ppT.tile([128, C], f32, tag="t", name="vp")
                            for ch in range(3):
                                nc.tensor.matmul(
                                    out=ps[:tk, :],
                                    lhsT=xh1[:, ch, t0:t0 + tk],
                                    rhs=wv1[:, ch, :],
                                    start=(ch == 0), stop=(ch == 2))
                            nc.scalar.copy(
                                out=v1[:tk, j, :, :HD],
                                in_=ps[:tk, :].rearrange("p (h d) -> p h d",
                                                         h=H))
                        mt = sb1.tile([128, 5, N], bf16, tag="maskt", bufs=2)
                        # j=4 block: only 16 valid key rows; zero the slot so
                        # the identity-matmul mask-init never reads garbage
                        # (the DMA then fills rows :16)
                        nc.vector.memset(mt[:, 4, :], 0.0)
                        for j, tk in enumerate(TKS):
                            nc.sync.dma_start(
                                out=mt[:tk, j, :],
                                in_=dd["maskT"][s, j * 128:j * 128 + tk, :])
                        attn_nT = sb1.tile([128, 3, N], f32r, tag="attn_nT",
                                           bufs=2)
                        for h in range(H):
                            po = (h % 2) * 64
                            att = [ppP.tile([HD + 1, TQ], f32, tag="p",
                                            name=f"att{t}") for t in range(2)]
                            for j, tk in enumerate(TKS):
                                t0 = j * 128
                                for tq in range(2):
                                    q0 = tq * TQ
                                    st = ppS.tile([128, TQ], f32, tag="s",
                                                  name="st")
                                    # init with additive mask, then QK
                                    nc.tensor.matmul(
                                        out=st[:tk, :],
                                        lhsT=ident_b[:, :tk],
                                        rhs=mt[:, j, q0:q0 + TQ],
                                        start=True, stop=False)
                                    po = (h % 2) * 64
                                    nc.tensor.matmul(
                                        out=st[:tk, :],
                                        lhsT=qkT[po:po + 64, 3 + h // 2,
                                                 t0:t0 + tk],
                                        rhs=qkT[po:po + 64, h // 2,
                                                q0:q0 + TQ],
                                        start=False, stop=True)
                                    ex = sb3.tile([128, TQ], bf16, tag="ex",
                                                  name="ex", bufs=3)
                                    nc.scalar.activation(out=ex[:tk, :],
                                                         in_=st[:tk, :],
                                                         func=ACTF.Exp)
                                    nc.tensor.matmul(
                                        out=att[tq][:],
                                        lhsT=v1[:tk, j, h, :],
                                        rhs=ex[:tk, :],
                                        start=(j == 0), stop=(j == 4))
                            for tq in range(2):
                                q0 = tq * TQ
                                rr = sb3.tile([1, TQ], f32r, tag="rr", bufs=2)
                                with nc.allow_low_precision(
                                        reason="softmax recip"):
                                    nc.vector.reciprocal(
                                        out=rr[:], in_=att[tq][HD:HD + 1, :])
                                # broadcast via PE (Pool queue stays free for
                                # the collectives)
                                bc = ppT.tile([64, TQ], f32, tag="t",
                                              name="bcq")
                                nc.tensor.matmul(out=bc[:],
                                                 lhsT=ones_row[:1, :64],
                                                 rhs=rr[:1, :], start=True,
                                                 stop=True)
                                bcs = sb3.tile([64, TQ], f32, tag="bcs",
                                               name="bcs", bufs=2)
                                nc.scalar.copy(out=bcs[:], in_=bc[:])
                                nc.vector.tensor_tensor(
                                    out=attn_nT[po:po + 64, h // 2,
                                                q0:q0 + TQ],
                                    in0=att[tq][:HD, :], in1=bcs[:],
                                    op=OP.mult)
                        x1T = x0T
                        for f in range(3):
                            for tq in range(2):
                                q0 = tq * TQ
                                ps = ppT.tile([128, TQ], f32, tag="t",
                                              name="pjp")
                                for ch in range(3):
                                    nc.tensor.matmul(
                                        out=ps[:],
                                        lhsT=wproj[:, ch,
                                                   f * 128:(f + 1) * 128],
                                        rhs=attn_nT[:, ch, q0:q0 + TQ],
                                        start=(ch == 0), stop=(ch == 2))
                                nc.vector.scalar_tensor_tensor(
                                    out=x1T[:, f, q0:q0 + TQ], in0=ps[:],
                                    scalar=cols["bproj"][:, f:f + 1],
                                    in1=x0T[:, f, q0:q0 + TQ],
                                    op0=OP.add, op1=OP.add)
                        xh2 = sb1.tile([128, 3, N], bf16, tag="xh2", bufs=2)
                        _std_fm(nc, sb3, ppT, x1T, N, ones_col_f, ones_row,
                                xh2)
                        xfnT = sb1.tile([128, 3, N], f32r, tag="xfnT", bufs=2)
                        for tq in range(2):
                            q0 = tq * TQ
                            h1T = sb1.tile([128, 12, TQ], bf16, tag="h1T",
                                           bufs=2)
                            for fh in range(12):
                                ps = ppT.tile([128, TQ], f32, tag="t",
                                              name="f1p")
                                for ch in range(3):
                                    nc.tensor.matmul(
                                        out=ps[:],
                                        lhsT=wfc1[:, ch,
                                                  fh * 128:(fh + 1) * 128],
                                        rhs=xh2[:, ch, q0:q0 + TQ],
                                        start=(ch == 0), stop=(ch == 2))
                                nc.scalar.activation(
                                    out=h1T[:, fh, :], in_=ps[:],
                                    func=ACTF.Gelu,
                                    bias=cols["bfc1"][:, fh:fh + 1],
                                    scale=1.0)
                            for f in range(3):
                                ps = ppT.tile([128, TQ], f32, tag="t",
                                              name="f2p")
                                for ch in range(12):
                                    nc.tensor.matmul(
                                        out=ps[:],
                                        lhsT=wfc2[:, ch,
                                                  f * 128:(f + 1) * 128],
                                        rhs=h1T[:, ch, :],
                                        start=(ch == 0), stop=(ch == 11))
                                nc.scalar.activation(
                                    out=xfnT[:, f, q0:q0 + TQ], in_=ps[:],
                                    func=ACTF.Identity,
                                    bias=cols["bfc2"][:, f:f + 1], scale=1.0)
                        x2T = sb1.tile([128, 3, N], f32, tag="x2T", bufs=2)
                        for tq in range(2):
                            q0 = tq * TQ
                            psd = ppT.tile([R, TQ], f32, tag="t", name="adp")
                            for ch in range(3):
                                nc.tensor.matmul(out=psd[:],
                                                 lhsT=wadd[:, ch, :],
                                                 rhs=xfnT[:, ch, q0:q0 + TQ],
                                                 start=(ch == 0),
                                                 stop=(ch == 2))
                            d0 = sb3.tile([R, TQ], f32r, tag="d0", name="d0",
                                          bufs=2)
                            nc.scalar.activation(out=d0[:], in_=psd[:],
                                                 func=ACTF.Gelu,
                                                 bias=adb_c[:, :1], scale=1.0)
                            for f in range(3):
                                psu = ppT.tile([128, TQ], f32, tag="t",
                                               name="aup")
                                nc.tensor.matmul(
                                    out=psu[:],
                                    lhsT=wadu[:, f * 128:(f + 1) * 128],
                                    rhs=d0[:], start=True, stop=True)
                                tt = sb3.tile([128, TQ], f32, tag="adt",
                                              name="tt", bufs=2)
                                nc.vector.scalar_tensor_tensor(
                                    out=tt[:], in0=psu[:],
                                    scalar=cols["adub"][:, f:f + 1],
                                    in1=xfnT[:, f, q0:q0 + TQ],
                                    op0=OP.add, op1=OP.add)
                                nc.vector.scalar_tensor_tensor(
                                    out=x2T[:, f, q0:q0 + TQ], in0=tt[:],
                                    scalar=gate_c[:, :1],
                                    in1=x1T[:, f, q0:q0 + TQ],
                                    op0=OP.mult, op1=OP.add)
                        nc.sync.dma_start(out=x2d[s, :, :, :], in_=x2T[:])
                        # ---- table rows: [raw | standardized] bf16 ----
                        x2b = sb1.tile([128, 3, 640], bf16, tag="x2b", bufs=2)
                        nc.vector.memset(x2b[:, :, N:], 0.0)
                        nc.vector.tensor_copy(out=x2b[:, :, :N], in_=x2T[:])
                        for j, tk in enumerate(TKS):
                            t0 = j * 128
                            tm = sb3.tile([128, RW], bf16, tag="tm", name="tm",
                                          bufs=3)
                            for ch in range(3):
                                nc.sync.dma_start_transpose(
                                    out=tm[:, ch * 128:(ch + 1) * 128],
                                    in_=x2b[:, ch, t0:t0 + 128])
                            st6 = sb3.tile([128, 6], f32, tag="st6", bufs=2)
                            nc.vector.bn_stats(out=st6[:], in_=tm[:, :C])
                            mv = sb3.tile([128, 2], f32, tag="mv", bufs=2)
                            nc.vector.bn_aggr(out=mv[:], in_=st6[:])
                            sd = sb3.tile([128, 1], f32, tag="sd", bufs=2)
                            nc.scalar.activation(out=sd[:], in_=mv[:, 1:2],
                                                 func=ACTF.Sqrt,
                                                 bias=eps_c[:, :1], scale=1.0)
                            rsd = sb3.tile([128, 1], f32, tag="rsd", bufs=2)
                            nc.vector.reciprocal(out=rsd[:], in_=sd[:])
                            # mean/std side-cols, then standardize in place
                            nc.vector.tensor_copy(out=tm[:, C:C + 1],
                                                  in_=mv[:, 0:1])
                            nc.vector.tensor_copy(out=tm[:, C + 1:C + 2],
                                                  in_=sd[:, 0:1])
                            nc.vector.memset(tm[:, C + 2:], 0.0)
                            nc.vector.tensor_scalar(
                                out=tm[:, :C], in0=tm[:, :C],
                                scalar1=mv[:, 0:1], scalar2=rsd[:, 0:1],
                                op0=OP.subtract, op1=OP.mult)
                            nc.sync.dma_start(
                                out=in_b[s][t0:t0 + tk, :], in_=tm[:tk, :])
                        # per-sample AllGather piece (overlaps next samples),
                        # then copy into the flat gather table
                        nc.gpsimd.collective_compute(
                            "AllGather", OP.bypass,
                            replica_groups=[list(range(N_CORES))],
                            ins=[in_b[s][:, :].opt()],
                            outs=[tbl_s[s][:, :].opt()])
                        nc.sync.dma_start(
                            out=table[s * TBLS:(s + 1) * TBLS, :],
                            in_=tbl_s[s][:, :])

                if "d_x2" in dbg and rep == 0:
                    for s in range(BPC):
                        nc.sync.dma_start(
                            out=dbg["d_x2"][s, :, :, :],
                            in_=x2d[s, :, :, :].rearrange("p a x -> a p x"))

                # ================= STAGE 2+3 =================
                with tc.tile_pool(name="sb2", bufs=1) as sb2, \
                     tc.tile_pool(name="sb4", bufs=3) as sb4, \
                     tc.tile_pool(name="pp2t", bufs=3, space="PSUM") as ppT, \
                     tc.tile_pool(name="pp2p", bufs=3, space="PSUM") as ppP, \
                     tc.tile_pool(name="pp2s", bufs=2, space="PSUM") as ppS:
                    # propagate weights (input-only) computed up front: this
                    # work fills the idle window while the last AllGather
                    # pieces drain
                    wTs = []
                    for s in range(BPC):
                        c2t = sb2.tile([3, S], f32, tag="c2t", bufs=2)
                        nc.sync.dma_start(out=c2t[:], in_=dd["c2T"][s, :, :])
                        c1t = sb2.tile([3, G], f32, tag="c1t", bufs=2)
                        nc.sync.dma_start(out=c1t[:], in_=dd["c1T"][s, :, :])
                        n1r = sb2.tile([1, G], f32, tag="n1r", bufs=2)
                        nc.sync.dma_start(out=n1r[:], in_=dd["n1sq"][s, :, :])
                        n1rr = sb2.tile([1, G], f32r, tag="n1rr", bufs=2)
                        nc.vector.tensor_copy(out=n1rr[:], in_=n1r[:])
                        n2c = sb2.tile([S, 1], f32, tag="n2c", bufs=2)
                        nc.sync.dma_start(out=n2c[:], in_=dd["n2sq"][s, :, :])
                        psd = ppP.tile([S, G], f32, tag="p", name="dps")
                        nc.tensor.matmul(out=psd[:], lhsT=c2t[:3, :],
                                         rhs=c1t[:3, :], start=True, stop=True)
                        nb = ppT.tile([128, G], f32, tag="t", name="nbp")
                        nc.tensor.matmul(out=nb[:], lhsT=ones_row[:1, :],
                                         rhs=n1rr[:1, :], start=True,
                                         stop=True)
                        nbs = sb2.tile([S, G], f32, tag="nbs", bufs=2)
                        nc.scalar.copy(out=nbs[:], in_=nb[:S, :])
                        dT = sb2.tile([S, G], f32, tag="dT", bufs=2)
                        nc.vector.scalar_tensor_tensor(
                            out=dT[:], in0=psd[:], scalar=-2.0, in1=nbs[:],
                            op0=OP.mult, op1=OP.add)
                        nc.vector.tensor_scalar(out=dT[:], in0=dT[:],
                                                scalar1=n2c[:, :1],
                                                scalar2=None, op0=OP.add)
                        rT = sb2.tile([S, G], f32r, tag="rT", bufs=2)
                        with nc.allow_low_precision(reason="propagate recip"):
                            nc.vector.reciprocal(out=rT[:], in_=dT[:])
                        pss = ppP.tile([1, G], f32, tag="p", name="rsum")
                        nc.tensor.matmul(out=pss[:], lhsT=ones_col[:S, :1],
                                         rhs=rT[:], start=True, stop=True)
                        rs = sb2.tile([1, G], f32r, tag="rs", bufs=2)
                        with nc.allow_low_precision(reason="propagate recip"):
                            nc.vector.reciprocal(out=rs[:], in_=pss[:])
                        rb = ppT.tile([128, G], f32, tag="t", name="rbp")
                        nc.tensor.matmul(out=rb[:], lhsT=ones_row[:1, :],
                                         rhs=rs[:1, :], start=True, stop=True)
                        wT = sb2.tile([S, G], f32r, tag="wTn", bufs=4)
                        nc.vector.tensor_tensor(out=wT[:], in0=rT[:],
                                                in1=rb[:S, :], op=OP.mult)
                        wTs.append(wT)
                    for s in range(BPC):
                        xc = sb2.tile([S, RW], bf16, tag="xc", bufs=2)
                        nc.gpsimd.indirect_dma_start(
                            out=xc[:], out_offset=None, in_=table[:],
                            in_offset=bass.IndirectOffsetOnAxis(
                                ap=cidx2[:, s:s + 1], axis=0))
                        xcT = sb2.tile([128, 3, S], bf16, tag="xcT", bufs=2)
                        msc = sb2.tile([2, S], bf16, tag="msc", bufs=2)
                        ptc = ppT.tile([2, 128], bf16, tag="t", name="ptc")
                        nc.tensor.transpose(out=ptc[:], in_=xc[:, C:C + 2],
                                            identity=ident_b[:])
                        nc.vector.tensor_copy(out=msc[:], in_=ptc[:])
                        csrow = sb2.tile([1, S], bf16, tag="csrow", bufs=2)
                        nc.sync.dma_start(out=csrow[:], in_=msc[1:2, :])
                        cmean_b = sb2.tile([128, S], bf16, tag="cmean_b",
                                           bufs=2)
                        nc.gpsimd.partition_broadcast(
                            cmean_b[:], msc[:1, :], channels=128)
                        cstd_b = sb2.tile([128, S], bf16, tag="cstd_b",
                                          bufs=2)
                        nc.gpsimd.partition_broadcast(
                            cstd_b[:], csrow[:1, :], channels=128)
                        for ch in range(3):
                            pt = ppT.tile([128, 128], bf16, tag="t",
                                          name="xcp")
                            nc.tensor.transpose(
                                out=pt[:], in_=xc[:, ch * 128:(ch + 1) * 128],
                                identity=ident_b[:])
                            tc2 = sb2.tile([128, S], bf16, tag="tc2", bufs=2)
                            nc.vector.tensor_tensor(out=tc2[:], in0=pt[:],
                                                    in1=cstd_b[:], op=OP.mult)
                            nc.vector.tensor_tensor(out=xcT[:, ch, :],
                                                    in0=tc2[:], in1=cmean_b[:],
                                                    op=OP.add)
                        vispre = sb2.tile([128, 3, S], f32, tag="vispre",
                                          bufs=2)

                        for cki in range(8):
                            gbase = (s * 8 + cki) * 4
                            xhT = sb2.tile([128, 3, 512], bf16, tag="xhT",
                                           bufs=3)
                            ms = sb4.tile([2, 512], bf16, tag="ms",
                                          name="ms", bufs=2)
                            for sub in range(4):
                                g2 = sb4.tile([128, RW], bf16, tag="g2",
                                              name="g2", bufs=4)
                                nc.gpsimd.indirect_dma_start(
                                    out=g2[:], out_offset=None, in_=table[:],
                                    in_offset=bass.IndirectOffsetOnAxis(
                                        ap=idx2[:, gbase + sub:gbase + sub + 1],
                                        axis=0))
                                o0 = sub * 128
                                for ch in range(3):
                                    pt = ppT.tile([128, 128], bf16, tag="t",
                                                  name="gtp")
                                    nc.tensor.transpose(
                                        out=pt[:],
                                        in_=g2[:, ch * 128:(ch + 1) * 128],
                                        identity=ident_b[:])
                                    if ch == 0:
                                        nc.vector.tensor_copy(
                                            out=xhT[:, ch, o0:o0 + 128],
                                            in_=pt[:])
                                    else:
                                        nc.scalar.copy(
                                            out=xhT[:, ch, o0:o0 + 128],
                                            in_=pt[:])
                                # mean/std cols -> 2 rows of ms
                                ptm = ppT.tile([2, 128], bf16, tag="t",
                                               name="ptm")
                                nc.tensor.transpose(
                                    out=ptm[:], in_=g2[:, C:C + 2],
                                    identity=ident_b[:])
                                nc.scalar.copy(out=ms[:, o0:o0 + 128],
                                               in_=ptm[:])
                            # std row (partition 1) -> partition 0 via DMA
                            srow = sb4.tile([1, 512], bf16, tag="srow",
                                            name="srow", bufs=2)
                            nc.sync.dma_start(out=srow[:], in_=ms[1:2, :])
                            # reconstruct raw rows: g = xh*std + mean
                            # (mean lands in the proj psum via a PE matmul)
                            std_b = sb4.tile([128, 512], bf16, tag="std_b",
                                             name="std_b", bufs=2)
                            nc.gpsimd.partition_broadcast(
                                std_b[:], srow[:1, :], channels=128)
                            tg3 = sb4.tile([128, 3, 512], bf16, tag="tg",
                                           name="tg", bufs=2)
                            for ch in range(3):
                                nc.gpsimd.tensor_tensor(
                                    out=tg3[:, ch, :], in0=xhT[:, ch, :],
                                    in1=std_b[:], op=OP.mult)
                            # head-pair packed q and k (slots 0-2 q, 3-5 k)
                            qk2 = sb2.tile([128, 6, 512], bf16, tag="qk2",
                                           bufs=3)
                            for qk in range(2):
                                for hp in range(3):
                                    ps = ppP.tile([128, 512], f32, tag="p",
                                                  name="qk2p")
                                    for ch in range(3):
                                        nc.tensor.matmul(
                                            out=ps[:],
                                            lhsT=wqk2[:, ch,
                                                      qk * C + hp * 128:
                                                      qk * C + (hp + 1) * 128],
                                            rhs=xhT[:, ch, :],
                                            start=(ch == 0), stop=(ch == 2))
                                    nc.scalar.activation(
                                        out=qk2[:, qk * 3 + hp, :], in_=ps[:],
                                        func=ACTF.Identity,
                                        bias=qb2c[:, qk * 3 + hp:
                                                  qk * 3 + hp + 1], scale=1.0)
                            v2 = sb2.tile([128, 4, H, HD + 1], bf16, tag="v2",
                                          bufs=2)
                            nc.vector.memset(v2[:, :, :, HD:HD + 1], 1.0)
                            for sub in range(4):
                                ps = ppT.tile([128, C], f32, tag="t",
                                              name="v2p")
                                for ch in range(3):
                                    nc.tensor.matmul(
                                        out=ps[:],
                                        lhsT=xhT[:, ch, sub * 128:
                                                 sub * 128 + 128],
                                        rhs=wv2[:, ch, :],
                                        start=(ch == 0), stop=(ch == 2))
                                nc.scalar.copy(
                                    out=v2[:, sub, :, :HD],
                                    in_=ps[:].rearrange("p (h d) -> p h d",
                                                        h=H))
                            at2 = sb2.tile([128, 3, 512], bf16, tag="at2",
                                           bufs=3)
                            for h in range(H):
                                po = (h % 2) * 64
                                stb = ppS.tile([128, 512], f32, tag="s",
                                               name="st2")
                                # init psum with the additive block mask
                                nc.tensor.matmul(
                                    out=stb[:], lhsT=ident_b[:, :],
                                    rhs=bmask[:, :], start=True, stop=False)
                                for sub in range(4):
                                    o0 = sub * 128
                                    nc.tensor.matmul(
                                        out=stb[:, o0:o0 + 128],
                                        lhsT=qk2[po:po + 64, 3 + h // 2,
                                                 o0:o0 + 128],
                                        rhs=qk2[po:po + 64, h // 2,
                                                o0:o0 + 128],
                                        start=False, stop=True)
                                ex = sb4.tile([128, 512], bf16, tag="ex2",
                                              name="ex2", bufs=2)
                                nc.scalar.activation(out=ex[:], in_=stb[:],
                                                     func=ACTF.Exp)
                                att = ppP.tile([HD + 1, 512], f32, tag="p",
                                               name="att2")
                                for sub in range(4):
                                    o0 = sub * 128
                                    nc.tensor.matmul(
                                        out=att[:, o0:o0 + 128],
                                        lhsT=v2[:, sub, h, :],
                                        rhs=ex[:, o0:o0 + 128],
                                        start=True, stop=True)
                                rr = sb4.tile([1, 512], bf16, tag="rr2",
                                              name="rr2", bufs=2)
                                with nc.allow_low_precision(
                                        reason="softmax recip"):
                                    nc.vector.reciprocal(
                                        out=rr[:], in_=att[HD:HD + 1, :])
                                bcs = sb4.tile([64, 512], bf16, tag="bcs2",
                                               name="bcs2", bufs=2)
                                nc.gpsimd.partition_broadcast(
                                    bcs[:], rr[:1, :], channels=64)
                                nc.vector.tensor_tensor(
                                    out=at2[po:po + 64, h // 2, :],
                                    in0=att[:HD, :], in1=bcs[:], op=OP.mult)
                            c0 = cki * 16
                            pm3 = sb4.tile([128, 3, 16], f32, tag="pm",
                                           bufs=2)
                            pa3 = sb4.tile([128, 3, 16], f32, tag="pa",
                                           bufs=2)
                            for f in range(3):
                                ps = ppP.tile([128, 512], f32, tag="p",
                                              name="pj2")
                                # accumulate the gathered-row mean into the
                                # proj psum (raw-row reconstruction)
                                nc.tensor.matmul(
                                    out=ps[:], lhsT=ones_row_b[:1, :],
                                    rhs=ms[:1, :], start=True, stop=False)
                                for ch in range(3):
                                    nc.tensor.matmul(
                                        out=ps[:],
                                        lhsT=wa1proj[:, ch,
                                                     f * 128:(f + 1) * 128],
                                        rhs=at2[:, ch, :], start=False,
                                        stop=(ch == 2))
                                xnn = sb4.tile([128, 512], f32, tag="xnn",
                                               name="xnn", bufs=2)
                                nc.vector.scalar_tensor_tensor(
                                    out=xnn[:], in0=ps[:],
                                    scalar=cols["ba1proj"][:, f:f + 1],
                                    in1=tg3[:, f, :], op0=OP.add, op1=OP.add)
                                nc.vector.tensor_reduce(
                                    out=pm3[:, f, :],
                                    in_=xnn[:].rearrange("p (g k) -> p g k",
                                                         k=GK),
                                    axis=AX.X, op=OP.max)
                                nc.vector.tensor_reduce(
                                    out=pa3[:, f, :],
                                    in_=xnn[:].rearrange("p (g k) -> p g k",
                                                         k=GK),
                                    axis=AX.X, op=OP.add)
                            nc.vector.scalar_tensor_tensor(
                                out=vispre[:, :, c0:c0 + 16], in0=pa3[:],
                                scalar=1.0 / GK, in1=pm3[:],
                                op0=OP.mult, op1=OP.add)

                        # deferred: bn + gelu + center mix (per sample)
                        vis_xT = sb2.tile([128, 3, S], f32, tag="vis_xT",
                                          bufs=2)
                        for f in range(3):
                            vg = sb4.tile([128, S], f32, tag="vg", bufs=2)
                            nc.scalar.activation(
                                out=vg[:], in_=vispre[:, f, :],
                                func=ACTF.Gelu,
                                bias=cols["bnbias"][:, f:f + 1],
                                scale=cols["bnscale"][:, f:f + 1])
                            nc.vector.scalar_tensor_tensor(
                                out=vis_xT[:, f, :], in0=xcT[:, f, :],
                                scalar=0.4, in1=vg[:], op0=OP.mult,
                                op1=OP.add)

                        # ---- stage 3 ----
                        vis_x = sb2.tile([128, 3, S], f32r, tag="vis_x",
                                         bufs=2)
                        for ch in range(3):
                            pt = ppT.tile([128, 128], f32, tag="t",
                                          name="vtp")
                            nc.tensor.transpose(out=pt[:],
                                                in_=vis_xT[:, ch, :],
                                                identity=ident[:])
                            nc.vector.tensor_copy(out=vis_x[:, ch, :],
                                                  in_=pt[:])
                        if "d_vis" in dbg and rep == 0:
                            for ch in range(3):
                                ot = sb4.tile([128, S], f32, tag="dbv",
                                              name="dbv", bufs=2)
                                nc.vector.tensor_copy(out=ot[:],
                                                      in_=vis_x[:, ch, :])
                                nc.sync.dma_start(
                                    out=dbg["d_vis"][s, ch, :, :], in_=ot[:])
                        wT = wTs[s]
                        xgT = sb2.tile([128, 3, G], f32, tag="xgT", bufs=2)
                        nc.sync.dma_start(out=xgT[:], in_=x2d[s, :, :, T:])
                        nxT = sb2.tile([128, 3, G], f32r, tag="nxT", bufs=2)
                        for ch in range(3):
                            psi = ppT.tile([128, G], f32, tag="t", name="ips")
                            nc.tensor.matmul(out=psi[:], lhsT=vis_x[:, ch, :],
                                             rhs=wT[:], start=True, stop=True)
                            nc.vector.scalar_tensor_tensor(
                                out=nxT[:, ch, :], in0=psi[:], scalar=0.4,
                                in1=xgT[:, ch, :], op0=OP.mult, op1=OP.add)
                        psa = ppT.tile([R, G], f32, tag="t", name="a1dp")
                        for ch in range(3):
                            nc.tensor.matmul(out=psa[:], lhsT=wa1d[:, ch, :],
                                             rhs=nxT[:, ch, :],
                                             start=(ch == 0), stop=(ch == 2))
                        d1 = sb2.tile([R, G], f32r, tag="d1", bufs=2)
                        nc.scalar.activation(out=d1[:], in_=psa[:],
                                             func=ACTF.Gelu,
                                             bias=a1db_c[:, :1], scale=1.0)
                        for ch in range(3):
                            psu = ppT.tile([128, G], f32, tag="t", name="a1up")
                            nc.tensor.matmul(
                                out=psu[:],
                                lhsT=wa1u[:, ch * 128:(ch + 1) * 128],
                                rhs=d1[:], start=True, stop=True)
                            oT = sb4.tile([128, G], f32, tag="oT", name="oT",
                                          bufs=2)
                            nc.vector.scalar_tensor_tensor(
                                out=oT[:], in0=psu[:],
                                scalar=cols["a1ub"][:, ch:ch + 1],
                                in1=nxT[:, ch, :], op0=OP.add, op1=OP.add)
                            for j in range(4):
                                pt = ppT.tile([128, 128], f32, tag="t",
                                              name="otp")
                                nc.tensor.transpose(
                                    out=pt[:], in_=oT[:, j * 128:(j + 1) * 128],
                                    identity=ident[:])
                                ob = sb4.tile([128, 128], f32, tag="ob",
                                              name="ob", bufs=2)
                                nc.scalar.copy(out=ob[:], in_=pt[:])
                                nc.sync.dma_start(
                                    out=y[s, j * 128:(j + 1) * 128,
                                          ch * 128:(ch + 1) * 128],
                                    in_=ob[:])

    nc.compile()
    return nc


_CACHE = {}


def _get_nc(repeat=1):
    key = f"nc{repeat}"
    if key not in _CACHE:
        _CACHE[key] = build(repeat=repeat)
    return _CACHE[key]


def prep_inputs(inputs):
    """Host-side prep: shard over batch, fold LN affines into weights."""
    xx = {k: np.asarray(v) for k, v in inputs.items()}
    bf = ml_dtypes.bfloat16
    shared = {}

    # stage-1 qkv with n1 affine folded; q-block pre-scaled by 1/sqrt(hd).
    # natural column layout is already head-pair packed: cols hp*128 hold
    # heads (2hp | 2hp+1) for q (0:384) and k (384:768).
    W1 = xx["qkv_w"] * xx["n1_g"][None, :]           # [1152, 384]
    qb_full = xx["n1_b"] @ xx["qkv_w"].T             # [1152]
    W1T = np.ascontiguousarray(W1.T)                 # [384, 1152]
    wqk1 = W1T[:, :2 * C].copy()
    wqk1[:, :C] *= SCALE
    qb1 = np.empty((128, 6), np.float32)
    qb1[:, 0:3] = (qb_full[:C] * SCALE).reshape(3, 128).T
    qb1[:, 3:6] = qb_full[C:2 * C].reshape(3, 128).T
    shared["wqk1"] = wqk1.reshape(3, 128, 768)
    shared["qb1"] = qb1
    shared["wv1"] = W1T[:, 2 * C:].reshape(3, 128, C)
    vb1 = qb_full[2 * C:]
    shared["wprojT"] = np.ascontiguousarray(xx["proj_w"].T).reshape(3, 128, C)
    shared["bproj"] = (xx["proj_b"] + xx["proj_w"] @ vb1).reshape(3, 128)

    # MLP with n2 affine folded
    F1 = xx["fc1_w"] * xx["n2_g"][None, :]
    shared["wfc1T"] = np.ascontiguousarray(F1.T).reshape(3, 128, H4).astype(bf)
    shared["bfc1"] = (xx["fc1_b"] + xx["fc1_w"] @ xx["n2_b"]).reshape(12, 128)
    shared["wfc2T"] = np.ascontiguousarray(xx["fc2_w"].T).reshape(
        12, 128, C).astype(bf)
    shared["bfc2"] = xx["fc2_b"].reshape(3, 128)

    shared["waddT"] = np.ascontiguousarray(xx["ad_dw"].T).reshape(3, 128, R)
    shared["adb"] = xx["ad_db"].reshape(R, 1)
    shared["waduT"] = np.ascontiguousarray(xx["ad_uw"].T).reshape(R, C)
    shared["adub"] = xx["ad_ub"].reshape(3, 128)
    shared["wa1dT"] = np.ascontiguousarray(xx["ad1_dw"].T).reshape(3, 128, R)
    shared["a1db"] = xx["ad1_db"].reshape(R, 1)
    shared["wa1uT"] = np.ascontiguousarray(xx["ad1_uw"].T).reshape(R, C)
    shared["a1ub"] = xx["ad1_ub"].reshape(3, 128)

    # stage-2 group attention with n3 affine folded (same packing)
    W2 = xx["a1_qkv_w"] * xx["n3_g"][None, :]
    qb2_full = xx["n3_b"] @ xx["a1_qkv_w"].T
    W2T = np.ascontiguousarray(W2.T)
    wqk2 = W2T[:, :2 * C].copy()
    wqk2[:, :C] *= SCALE
    qb2 = np.empty((128, 6), np.float32)
    qb2[:, 0:3] = (qb2_full[:C] * SCALE).reshape(3, 128).T
    qb2[:, 3:6] = qb2_full[C:2 * C].reshape(3, 128).T
    shared["wqk2"] = wqk2.reshape(3, 128, 768).astype(bf)
    shared["qb2"] = qb2
    shared["wv2"] = W2T[:, 2 * C:].reshape(3, 128, C).astype(bf)
    vb2 = qb2_full[2 * C:]
    shared["wa1projT"] = np.ascontiguousarray(xx["a1_proj_w"].T).reshape(
        3, 128, C).astype(bf)
    shared["ba1proj"] = (xx["a1_proj_b"] + xx["a1_proj_w"] @ vb2).reshape(
        3, 128)

    shared["bnscale"] = (xx["bn_g"] / np.sqrt(np.float32(1.0 + 1e-5))
                         ).reshape(3, 128)
    shared["bnbias"] = xx["bn_b"].reshape(3, 128)
    shared["gate"] = xx["ad_gate"].reshape(1, 1)
    bm = np.full((128, 128), NEG, np.float32)
    for g in range(4):
        bm[g * 32:(g + 1) * 32, g * 32:(g + 1) * 32] = 0.0
    shared["blockmask"] = np.tile(bm, (1, 4)).astype(bf)
    shared = {k: np.ascontiguousarray(v) for k, v in shared.items()}

    prompt = xx["prompt_embeddings"]
    # remap gather indices to the sample-major table layout:
    # old row = b*528 + t  ->  new row = (b%4)*4224 + (b//4)*528 + t
    def remap(ix):
        b = ix // N
        t = ix % N
        return ((b % BPC) * TBLS + (b // BPC) * N + t).astype(np.int32)

    idx = remap(xx["idx"])
    cidx = remap(xx["center_idx"])
    idx2 = np.ascontiguousarray(idx.reshape(B, S * GK))
    cidx2 = np.ascontiguousarray(cidx.reshape(B, S))

    maps = []
    for c in range(N_CORES):
        sl = slice(c * BPC, (c + 1) * BPC)
        xin = np.concatenate(
            [np.broadcast_to(prompt[None], (BPC, T, C)), xx["x"][sl]], axis=1)
        m = {}
        m["xinT"] = np.ascontiguousarray(xin.transpose(0, 2, 1)).reshape(
            BPC, 3, 128, N)
        mp = np.zeros((BPC, N, N), np.float32)
        mp[:, :G, :G] = NEG * xx["mask"][sl]
        m["maskT"] = np.ascontiguousarray(mp.transpose(0, 2, 1)).astype(bf)
        # gather-block columns: column g holds rows g*128..g*128+127
        m["idx2"] = np.ascontiguousarray(
            idx2[sl].reshape(-1).reshape(BPC * S * GK // 128, 128).T)
        m["cidx2"] = np.ascontiguousarray(
            cidx2[sl].reshape(-1).reshape(BPC, 128).T)
        c1 = xx["center1"][sl]
        c2 = xx["center2"][sl]
        m["c1T"] = np.ascontiguousarray(c1.transpose(0, 2, 1))
        m["n1sq"] = np.ascontiguousarray((c1 ** 2).sum(-1)[:, None, :])
        m["c2T"] = np.ascontiguousarray(c2.transpose(0, 2, 1))
        m["n2sq"] = np.ascontiguousarray((c2 ** 2).sum(-1)[:, :, None] + 1e-8)
        m.update(shared)
        maps.append({k: np.ascontiguousarray(v) for k, v in m.items()})
    return maps


def run(maps, nc=None):
    if nc is None:
        nc = _get_nc()
    res = run_bass_kernel_spmd(nc, maps, core_ids=list(range(N_CORES)))
    return res.results


def kernel(**inputs):
    maps = prep_inputs(inputs)
    results = run(maps)
    out = np.concatenate([r["y"] for r in results], axis=0)
    return out.astype(np.float32)
